# revision 44
# baseline (speedup 1.0000x reference)
"""WENO5 2D advection (Advection3D) Trainium2 kernel — bf16 compute with
float16 flux tail.

Full inputs h, u, v: [32, 1024, 1024] f32.  Output: same shape f32;
out[1:-1, 2:-2, 2:-2] = -div(WENO5 fluxes), 0 on the frame.

Work decomposition: the 30 interior z-levels x 9 y-chunks = 270
independent 128-row units are distributed flat across 8 cores (34 units
per core, 2 dummy repeats) — no per-core z padding waste.

Perf design (v2; baseline notes in kernel_baseline.py):
  - h bf16 in SBUF; WENO smoothness/weight chain in bf16 (DVE 2x_1p
    packed; all APs 4B-aligned).  u, v are f16 (pre-scaled by 1/DX,
    1/DY on host): relu tensor_scalar runs 4x_2p.
  - rd = (5/6)/den via ONE ACT Reciprocal op (scale=1.2) instead of
    Ln+Exp; the whole kernel fits the `reciprocal_and_small` activation
    table set (reciprocal/copy/square/relu) — no table reloads.
  - y-direction R-side partition shifts ride the PE band matrices:
    B0m1/B2p1 come from SHM1/SHP1-shifted c-accumulations (the ACT
    Square IS the PSUM->SBUF transfer), and denRp1/numRp1 use
    SHP1-scaled accumulation bands on unshifted PP/g products.  Zero
    shifted-copy ACT ops remain (baseline had 6 per chunk).
  - Divergence combine on PE: psum = DFY@fn + I1f@fe(col-1) - I1f@fe
    accumulated f16 band matmuls; one ACT Copy PSUM->SBUF f32 writes
    the output tile.  (baseline: 2 fp32 DVE ops + an SBUF shift DMA.)

Math (per face i, L stored at i, R stored at i+1 ("primed"); D_j =
q_{j+1}-q_j, A_j = D_j - D_{j-1}):
  G0 = c1312 A^2 + (0.5A + D)^2        (Sq of t0h)
  G1 = c1312 A^2 + (0.5(D+DS))^2       (Sq of sh)
  G2 = c1312 A^2 + (0.5A - DS)^2       (Sq of t1h)
  B_k = (eps + G_k)^2 ; PP12 = B1*B2S, PP01 = B0S*B1, PP02 = B0S*B2S
  denL10 = PP12 + 6 PP02 + 3 PP01 ; denR10 = PP01 + 6 PP02 + 3 PP12
  rdL = (5/6)/denL10 = Reciprocal(1.2*denL10)
  numL12 = PP12*dl0L + PP02*(7.2 dl1L) + PP01*(7.2 dl2L)   (1.2x bands)
  qL = q_i + numL12*rdL ; qR' = q_j - numR12'*rdR'   (j = i+1)
  flux = relu(U)*qL - relu(-U)*qR
"""
import math

import numpy as np

import concourse.bass as bass
import concourse.mybir as mybir
import concourse.tile as tile

F32 = mybir.dt.float32
F16 = mybir.dt.float16
BF16 = mybir.dt.bfloat16
ALU = mybir.AluOpType
AF = mybir.ActivationFunctionType

NZ, NY, NX = 32, 1024, 1024
NCORES = 8
PY, PX = NY + 2, NX + 2      # edge-padded input
W = 1032                     # tile width; data at cols [2:1028) <-> padded [0:1026)
DX = 1000.0
DY = 1000.0
WENO_EPS = 1e-6
C1312 = 13.0 / 12.0
C1312S = math.sqrt(C1312)
LN56 = math.log(5.0 / 6.0)
RD_FAST = False              # rd = (5/6)/den via 1 custom-DVE op vs ACT ln/exp
CHUNK = 122                  # valid output rows per 128-row unit
R0S = [0, 122, 244, 366, 488, 610, 732, 854, 898]
UNITS_PER_CORE = 34          # ceil(30*9 / 8)


class LegalTileContext(tile.TileContext):
    """Tile + wait legalization: this walrus packs at most ONE semaphore wait
    per instruction; hoist extras onto standalone EventSemaphore instructions
    (what raw-bass wait_ge emits)."""

    def _commit_instruction(self, inst, lazy_reg_writes=True):
        si = inst.sync_info
        if si is not None and len(si.on_wait) > 1:
            waits = list(si.on_wait)
            for w in waits[:-1]:
                ev = mybir.InstEventSemaphore(
                    name=f"W-{self.nc.next_id()}", ins=[], outs=[]
                )
                ev.engine = inst.engine
                ev.sync_info = mybir.SyncInfo(on_wait=[w], on_update=[])
                if inst.debug is not None:
                    ev.debug = inst.debug
                super()._commit_instruction(ev, lazy_reg_writes=False)
            inst.sync_info = mybir.SyncInfo(
                on_wait=[waits[-1]], on_update=list(si.on_update)
            )
        return super()._commit_instruction(inst, lazy_reg_writes)

    def _drain_and_barrier(self, tick_clock, wait_clock):
        from concourse.vector_clock import ScopedClock

        nop0 = self.nc.sync.nop()
        wait_clock.add_sem_waits(
            nop0.ins, ScopedClock({None: tick_clock.global_clock})
        )
        si = nop0.ins.sync_info
        if si is not None and len(si.on_wait) > 1:
            waits = list(si.on_wait)
            nop0.ins.sync_info = mybir.SyncInfo(
                on_wait=[waits[0]], on_update=list(si.on_update)
            )
            for w in waits[1:]:
                nopk = self.nc.sync.nop()
                nopk.ins.sync_info = mybir.SyncInfo(on_wait=[w], on_update=[])
        self.nc.sync.drain()

        self.nc.all_engine_barrier()
        assert self.sems is not None
        popped = self.nc._tile_sem_poison_stack.pop()
        assert popped is self._sem_poison
        self.nc.clear_and_free_semaphores(list(self.sems.allocated().values()))
        self.nc.all_engine_barrier()


class Scratch:
    """Free-list scratch allocator.  Tags are reused only after an explicit
    free(), which callers place after the tile's last consumer is emitted —
    so slot-wait edges always point backward in emission order and can
    never form a scheduling cycle."""

    def __init__(self, pool, shape, dtype, prefix="s", prime=0):
        self.pool = pool
        self.shape = shape
        self.dtype = dtype
        self.prefix = prefix
        # prime: pre-seed k tags so consumers round-robin across several
        # slots (each slot double-buffered by the pool) instead of
        # serializing on one.
        self.free_tags = [f"{prefix}{i}" for i in range(prime)]
        self.n = prime
        self.tag_of = {}

    def __call__(self):
        # FIFO reuse: freed tags get maximal cool-down before their buffers
        # are written again (fewer WAR slot waits than LIFO).
        tag = (
            self.free_tags.pop(0) if self.free_tags else f"{self.prefix}{self._new()}"
        )
        t = self.pool.tile(self.shape, self.dtype, tag=tag)
        self.tag_of[id(t)] = tag
        return t

    def _new(self):
        self.n += 1
        return self.n - 1

    def free(self, *tiles):
        for t in tiles:
            self.free_tags.append(self.tag_of.pop(id(t)))


# Band matrices (lhsT layout: S[k, p] = coeff of q_k in out_p), bf16.
BAND_SPECS = [
    ("shp1", {1: 1.0}),                        # 0: out_p = q_{p+1}
    ("ay", {-1: 1.0, 0: -2.0, 1: 1.0}),        # 1: A_p
    ("t0h", {-1: 0.5, 0: -2.0, 1: 1.5}),       # 2: 0.5*A + D
    ("t1h", {-1: 1.5, 0: -2.0, 1: 0.5}),       # 3: 0.5*A - DS
    ("sh", {-1: -0.5, 1: 0.5}),                # 4: 0.5*(D + DS)
    ("dl0L", {-2: 0.4, -1: -1.4, 0: 1.0}),     # 5
    ("dl1Lh", {-1: -1.2, 0: -1.2, 1: 2.4}),    # 6: 7.2*dl1L
    ("dl2Lh", {0: -2.4, 1: 3.0, 2: -0.6}),     # 7: 7.2*dl2L
    ("dl0Rm", {0: -1.0, 1: 1.4, 2: -0.4}),     # 8: dl0R shifted -1
    ("dl1Rm", {-1: -2.4, 0: 1.2, 1: 1.2}),     # 9: 7.2*dl1R shifted -1
    ("dl2Rm", {-2: 0.6, -1: -3.0, 0: 2.4}),    # 10: 7.2*dl2R shifted -1
    ("shm1", {-1: 1.0}),                       # 11: out_p = q_{p-1}
    ("i1", {0: 1.0}),                          # 12: identity (accumulate)
    # den bands pre-scaled by 1.2 so rd = plain reciprocal of the PSUM sum:
    # (5/6)/den10 = 1/(1.2*den10)
    ("i12", {0: 1.2}),                         # 13
    ("i72", {0: 7.2}),                         # 14
    ("i36", {0: 3.6}),                         # 15
    ("shp12", {1: 1.2}),                       # 16
    ("shp72", {1: 7.2}),                       # 17
    ("shp36", {1: 3.6}),                       # 18
    # x num bands: absorb the dl ratio scales (dl' = dl/s, band = s)
    ("im04", {0: -0.4}),                       # 19
    ("im06", {0: -0.6}),                       # 20
]
SHP1, AY, T0H, T1H, SH = 0, 1, 2, 3, 4
DL0L, DL1LH, DL2LH, DL0RM, DL1RM, DL2RM = 5, 6, 7, 8, 9, 10
SHM1, I1, I12, I72, I36, SHP12, SHP72, SHP36 = 11, 12, 13, 14, 15, 16, 17, 18
IM04, IM06 = 19, 20
NBANDS = len(BAND_SPECS)
# f16 combine bands: psum = DFY@yaa - DFY@ybb + I1F@x(col-1) - I1F@x(col)
FBAND_SPECS = [
    ("dfy", {-1: 1.0, 0: -1.0}),               # 0: t_{p-1} - t_p
    ("i1f", {0: 1.0}),                         # 1
    ("im1f", {0: -1.0}),                       # 2
    ("dfym", {-1: -1.0, 0: 1.0}),              # 3: t_p - t_{p-1}
]
DFY, I1F, IM1F, DFYM = 0, 1, 2, 3
NFBANDS = len(FBAND_SPECS)


def _band_matrix(taps, edge_diag=False):
    w = np.zeros((128, 128), dtype=np.float32)
    for off, coef in taps.items():
        for p in range(128):
            k = p + off
            if 0 <= k < 128:
                w[k, p] = coef
    if edge_diag:
        # rows whose taps all fell out of range get a diagonal entry so
        # shifted den accumulations stay positive (reciprocal-safe).
        # Those rows are outside the valid output window.
        for p in range(128):
            if not w[:, p].any():
                w[p, p] = next(iter(taps.values()))
    return w


def make_bands_host():
    """SBUF-layout band matrices: [128, NBANDS*128] bf16."""
    import ml_dtypes

    w = np.zeros((128, NBANDS * 128), dtype=np.float32)
    for b, (name, taps) in enumerate(BAND_SPECS):
        w[:, b * 128 : (b + 1) * 128] = _band_matrix(
            taps, edge_diag=name.startswith("shp")
        )
    return w.astype(ml_dtypes.bfloat16)


def make_fbands_host():
    w = np.zeros((128, NFBANDS * 128), dtype=np.float32)
    for b, (_, taps) in enumerate(FBAND_SPECS):
        w[:, b * 128 : (b + 1) * 128] = _band_matrix(taps)
    return w.astype(np.float16)


E = slice(2, 1028)    # x-chain window (even start/len; data cols)
EY = slice(4, 1028)   # y-chain window (1024 cols = 2 PSUM banks)


def _emit_chunk(nc, sc, scf, sch, psc, bands, fbands, Q, Uf, Vf, oc2):
    """Emit one 128-row unit, x/y chains interleaved.

    sc: bf16 scratch; scf: fp32 scratch; sch: f16 scratch; psc: PSUM
    scratch.  Q bf16; Uf, Vf f16 (pre-scaled by 1/DX, 1/DY on host).
    Result (fp32) is written to oc2; valid rows [3:125), cols [5:1025).
    """
    tt = nc.vector.tensor_tensor
    tsm = nc.vector.tensor_scalar_mul
    act = nc.scalar.activation

    def pe(src, b, lo=4, bsrc=None):
        bsrc = bands if bsrc is None else bsrc
        pt = psc()
        for c0 in (0, 512):
            nc.tensor.matmul(
                pt[:, c0 : c0 + 512],
                bsrc[:, b * 128 : (b + 1) * 128],
                src[:, lo + c0 : lo + c0 + 512],
            )
        return pt

    def pe_acc(srcs_and_bands, lo, bsrc=None):
        """PSUM-accumulated sum of band-stencils: sum_k band_k @ src_k.
        Entries may be (src, band) with shared lo, or (src, band, lo)."""
        bsrc = bands if bsrc is None else bsrc
        pt = psc()
        n = len(srcs_and_bands)
        for c0 in (0, 512):
            for k, ent in enumerate(srcs_and_bands):
                src, b = ent[0], ent[1]
                l = ent[2] if len(ent) > 2 else lo
                nc.tensor.matmul(
                    pt[:, c0 : c0 + 512],
                    bsrc[:, b * 128 : (b + 1) * 128],
                    src[:, l + c0 : l + c0 + 512],
                    start=(k == 0),
                    stop=(k == n - 1),
                )
        return pt

    def pecopy(src, b, func=AF.Copy, scale=1.0):
        p = pe(src, b)
        t = sc()
        act(t[:, EY], p[:, 0:1024], func, scale=scale)
        psc.free(p)
        return t

    def rd_of(dp, win):
        """rd = (5/6)/den10 = 1/(1.2*den10); the 1.2 rides the den
        accumulation bands.  RD_FAST: one custom-DVE reciprocal (fp32,
        ~18 bits); else ACT Ln + Exp(-x)."""
        if RD_FAST:
            rdf = scf()
            nc.vector.reciprocal_approx_fast(rdf[:, win], dp[:, 0:1024])
            return rdf, scf
        # fp32 ln/rd tiles: ACT cost is dtype-independent, and the t-product
        # reads PSUM fp32 anyway (1x) — fp32 here is free accuracy.
        ln = scf()
        act(ln[:, win], dp[:, 0:1024], AF.Ln)
        rd = scf()
        act(rd[:, win], ln[:, win], AF.Exp, scale=-1.0)
        scf.free(ln)
        return rd, scf

    XL = slice(4, 1026)   # x late-section window (after PP)

    # ---- y producers: PE band stencils + ACT copies (need only Q).
    # R-side dl's and qs1 are emitted later, near their consumers, to
    # cap the bf16 scratch pool's concurrent-tag peak. ----
    yasq = pecopy(Q, AY, AF.Square, C1312S)
    yq0 = pecopy(Q, T0H, AF.Square)
    yq2 = pecopy(Q, T1H, AF.Square)
    yq1 = pecopy(Q, SH, AF.Square)
    ydl0L = pecopy(Q, DL0L)
    ydl1L = pecopy(Q, DL1LH)
    ydl2L = pecopy(Q, DL2LH)

    # ---- x stencils: QS/DS shifted copies on DVE (4x), diffs on DVE,
    # squares on ACT.  Guard memsets zero the few columns that stencil
    # windows read beyond the written region (values never reach the
    # valid output window; zeroing keeps NaN-free and exec-sim clean) ----
    nc.gpsimd.memset(Q[:, 1028:1030], 0.0)
    xQS = sc(); nc.sync.dma_start(xQS[:, E], Q[:, 3:1029])
    xD = sc(); tt(xD[:, E], xQS[:, E], Q[:, E], ALU.subtract)
    nc.gpsimd.memset(xD[:, 0:2], 0.0)
    xDS = sc(); nc.sync.dma_start(xDS[:, E], xD[:, 1:1027])
    xA = sc(); tt(xA[:, E], xD[:, E], xDS[:, E], ALU.subtract)
    xD05A = sc(); tsm(xD05A[:, E], xA[:, E], 0.5)
    xt0h = sc(); tt(xt0h[:, E], xD05A[:, E], xD[:, E], ALU.add)
    xt1h = sc(); tt(xt1h[:, E], xD05A[:, E], xDS[:, E], ALU.subtract)
    sc.free(xD05A)
    xs = sc(); tt(xs[:, E], xD[:, E], xDS[:, E], ALU.add)
    xasq = sc(); act(xasq[:, E], xA[:, E], AF.Square, scale=C1312S)
    xq0 = sc(); act(xq0[:, E], xt0h[:, E], AF.Square)
    xq1 = sc(); act(xq1[:, E], xs[:, E], AF.Square, scale=0.5)
    xq2 = sc(); act(xq2[:, E], xt1h[:, E], AF.Square)
    sc.free(xA)
    sc.free(xt0h, xt1h, xs)
    # pre-scaled D variants (tensor_scalar, 4x_2p at any alignment);
    # dl' = dl / s with s absorbed into the x num bands (IM04/I12/IM06):
    #   dl0L' = D[c-2] - 2.5 D[c-1]      (s = -0.4)
    #   dl1L' = DS + 2 D                 (s = 1.2)
    #   dl2L' = DS[c+2] - 4 D            (s = -0.6)   (R side mirrored)
    xDM25 = sc(); tsm(xDM25[:, E], xD[:, E], -2.5)
    xDM25S = sc(); tsm(xDM25S[:, E], xDS[:, E], -2.5)
    xD2 = sc(); tsm(xD2[:, E], xD[:, E], 2.0)
    xD2S = sc(); tsm(xD2S[:, E], xDS[:, E], 2.0)
    xDM4 = sc(); tsm(xDM4[:, E], xD[:, E], -4.0)
    xDM4S = sc(); tsm(xDM4S[:, E], xDS[:, E], -4.0)
    xdl0L = sc(); tt(xdl0L[:, XL], xD[:, 2:1024], xDM25S[:, XL], ALU.add)
    xdl1L = sc(); tt(xdl1L[:, XL], xDS[:, XL], xD2[:, XL], ALU.add)
    xdl2L = sc(); tt(xdl2L[:, XL], xDS[:, 6:1028], xDM4[:, XL], ALU.add)
    xdl0R = sc(); tt(xdl0R[:, XL], xDS[:, 6:1028], xDM25[:, XL], ALU.add)
    xdl1R = sc(); tt(xdl1R[:, XL], xD[:, XL], xD2S[:, XL], ALU.add)
    xdl2R = sc(); tt(xdl2R[:, XL], xD[:, 2:1024], xDM4S[:, XL], ALU.add)
    sc.free(xDM25, xDM25S, xD2, xD2S, xDM4, xDM4S, xD, xDS, xQS)

    # ---- y: B's from shifted c-accumulations (the Square IS the
    # PSUM->SBUF transfer; row shifts ride the SHM1/I1/SHP1 bands) ----
    ycp = pe_acc([(yasq, SHM1), (yq0, SHM1)], 4)
    yB0m1 = sc(); act(yB0m1[:, EY], ycp[:, 0:1024], AF.Square, bias=WENO_EPS)
    psc.free(ycp)
    ycp = pe_acc([(yasq, I1), (yq1, I1)], 4)
    yB1 = sc(); act(yB1[:, EY], ycp[:, 0:1024], AF.Square, bias=WENO_EPS)
    psc.free(ycp)
    ycp = pe_acc([(yasq, SHP1), (yq2, SHP1)], 4)
    yB2p1 = sc(); act(yB2p1[:, EY], ycp[:, 0:1024], AF.Square, bias=WENO_EPS)
    psc.free(ycp)
    sc.free(yasq, yq0, yq1, yq2)

    # ---- x: same, with the B shifts folded into the PSUM->SBUF writes
    # (c-psum col c <-> x col c+3) ----
    xcp = pe_acc([(xasq, I1), (xq0, I1)], 3)
    xB0S = sc()  # xB0S[t] = B0[t-1]
    act(xB0S[:, 4:1028], xcp[:, 0:1024], AF.Square, bias=WENO_EPS)
    psc.free(xcp)
    xcp = pe_acc([(xasq, I1), (xq1, I1)], 3)
    xB1 = sc()
    act(xB1[:, 3:1027], xcp[:, 0:1024], AF.Square, bias=WENO_EPS)
    psc.free(xcp)
    xcp = pe_acc([(xasq, I1), (xq2, I1)], 3)
    xB2S = sc()  # xB2S[t] = B2[t+1]
    act(xB2S[:, 2:1026], xcp[:, 0:1024], AF.Square, bias=WENO_EPS)
    psc.free(xcp)
    sc.free(xasq, xq0, xq1, xq2)

    # ---- y: PP products (DVE), dens on PE-accumulate (R-shift rides the
    # SHP bands), reciprocal (ACT) ----
    yPP12 = sc(); tt(yPP12[:, EY], yB1[:, EY], yB2p1[:, EY], ALU.mult)
    yPP01 = sc(); tt(yPP01[:, EY], yB0m1[:, EY], yB1[:, EY], ALU.mult)
    yPP02 = sc(); tt(yPP02[:, EY], yB0m1[:, EY], yB2p1[:, EY], ALU.mult)
    sc.free(yB1, yB0m1, yB2p1)
    yg0L = sc(); tt(yg0L[:, EY], yPP12[:, EY], ydl0L[:, EY], ALU.mult)
    yg1L = sc(); tt(yg1L[:, EY], yPP02[:, EY], ydl1L[:, EY], ALU.mult)
    yg2L = sc(); tt(yg2L[:, EY], yPP01[:, EY], ydl2L[:, EY], ALU.mult)
    sc.free(ydl0L, ydl1L, ydl2L)
    ydp = pe_acc([(yPP12, I12), (yPP02, I72), (yPP01, I36)], 4)
    yrdL, yrdL_sc = rd_of(ydp, EY)
    psc.free(ydp)
    ydp = pe_acc([(yPP01, SHP12), (yPP02, SHP72), (yPP12, SHP36)], 4)
    yrdR, yrdR_sc = rd_of(ydp, EY)
    psc.free(ydp)

    # ---- x: PP products, dens on PE (psum col c <-> x col c+2), rcp ----
    xPP12 = sc(); tt(xPP12[:, XL], xB1[:, XL], xB2S[:, XL], ALU.mult)
    xPP01 = sc(); tt(xPP01[:, XL], xB0S[:, XL], xB1[:, XL], ALU.mult)
    xPP02 = sc(); tt(xPP02[:, XL], xB0S[:, XL], xB2S[:, XL], ALU.mult)
    sc.free(xB1, xB0S, xB2S)
    # den/num accumulations read 2 guard cols before XL; keep them
    # finite (psum cols 0-1 are never consumed downstream)
    for _t in (xPP12, xPP01, xPP02):
        nc.gpsimd.memset(_t[:, 2:4], 1.0)
    xg0L = sc(); tt(xg0L[:, XL], xPP12[:, XL], xdl0L[:, XL], ALU.mult)
    xg1L = sc(); tt(xg1L[:, XL], xPP02[:, XL], xdl1L[:, XL], ALU.mult)
    xg2L = sc(); tt(xg2L[:, XL], xPP01[:, XL], xdl2L[:, XL], ALU.mult)
    sc.free(xdl0L, xdl1L, xdl2L)
    for _t in (xg0L, xg1L, xg2L):
        nc.gpsimd.memset(_t[:, 2:4], 1.0)
    xg0R = sc(); tt(xg0R[:, XL], xPP01[:, XL], xdl0R[:, XL], ALU.mult)
    xg1R = sc(); tt(xg1R[:, XL], xPP02[:, XL], xdl1R[:, XL], ALU.mult)
    xg2R = sc(); tt(xg2R[:, XL], xPP12[:, XL], xdl2R[:, XL], ALU.mult)
    sc.free(xdl0R, xdl1R, xdl2R)
    for _t in (xg0R, xg1R, xg2R):
        nc.gpsimd.memset(_t[:, 2:4], 1.0)
    xdp = pe_acc([(xPP12, I12), (xPP02, I72), (xPP01, I36)], 2)
    xrdL, xrdL_sc = rd_of(xdp, slice(2, 1026))
    psc.free(xdp)
    xdp = pe_acc([(xPP01, I12), (xPP02, I72), (xPP12, I36)], 2)
    xrdR, xrdR_sc = rd_of(xdp, slice(2, 1026))
    psc.free(xdp)
    sc.free(xPP12, xPP01, xPP02)

    # ---- y: R gammas (unshifted; the +1 shift rides the num bands),
    # nums, reconstruction, flux ----
    ydl0R = pecopy(Q, DL0RM)
    ydl1R = pecopy(Q, DL1RM)
    ydl2R = pecopy(Q, DL2RM)
    yg0R = sc(); tt(yg0R[:, EY], yPP01[:, EY], ydl0R[:, EY], ALU.mult)
    yg1R = sc(); tt(yg1R[:, EY], yPP02[:, EY], ydl1R[:, EY], ALU.mult)
    yg2R = sc(); tt(yg2R[:, EY], yPP12[:, EY], ydl2R[:, EY], ALU.mult)
    sc.free(yPP12, yPP01, yPP02, ydl0R, ydl1R, ydl2R)
    ynLp = pe_acc([(yg0L, I1), (yg1L, I1), (yg2L, I1)], 4)
    sc.free(yg0L, yg1L, yg2L)
    ytL = sch(); tt(ytL[:, EY], ynLp[:, 0:1024], yrdL[:, EY], ALU.mult)
    psc.free(ynLp)
    yrL = sch(); tt(yrL[:, EY], Q[:, EY], ytL[:, EY], ALU.add)
    yrdL_sc.free(yrdL); sch.free(ytL)
    ynRp = pe_acc([(yg0R, SHP1), (yg1R, SHP1), (yg2R, SHP1)], 4)
    sc.free(yg0R, yg1R, yg2R)
    ytR = sch(); tt(ytR[:, EY], ynRp[:, 0:1024], yrdR[:, EY], ALU.mult)
    psc.free(ynRp)
    pqs1 = pe(Q, SHP1)
    yrR = sch(); tt(yrR[:, EY], pqs1[:, 0:1024], ytR[:, EY], ALU.subtract)
    yrdR_sc.free(yrdR); psc.free(pqs1); sch.free(ytR)
    # relu(V), relu(-V): f16 tensor_scalar runs 4x_2p on DVE
    ypV = sch(); nc.vector.tensor_scalar_max(ypV[:, EY], Vf[:, EY], 0.0)
    ypVm = sch(); nc.vector.tensor_scalar(
        ypVm[:, EY], Vf[:, EY], -1.0, 0.0, ALU.mult, ALU.max)
    yaa = sch(); tt(yaa[:, EY], ypV[:, EY], yrL[:, EY], ALU.mult)
    sch.free(yrL, ypV)
    ybb = sch(); tt(ybb[:, EY], ypVm[:, EY], yrR[:, EY], ALU.mult)
    sch.free(ypVm, yrR)

    # ---- x: nums, reconstruction, flux (window XL) ----
    xnLp = pe_acc([(xg0L, IM04), (xg1L, I12), (xg2L, IM06)], 2)
    sc.free(xg0L, xg1L, xg2L)
    xtL = sch(); tt(xtL[:, XL], xnLp[:, 2:1024], xrdL[:, XL], ALU.mult)
    psc.free(xnLp)
    xrL = sch(); tt(xrL[:, XL], Q[:, XL], xtL[:, XL], ALU.add)
    xrdL_sc.free(xrdL); sch.free(xtL)
    xnRp = pe_acc([(xg0R, IM04), (xg1R, I12), (xg2R, IM06)], 2)
    sc.free(xg0R, xg1R, xg2R)
    xtR = sch(); tt(xtR[:, XL], xnRp[:, 2:1024], xrdR[:, XL], ALU.mult)
    psc.free(xnRp)
    xrR = sch(); tt(xrR[:, XL], Q[:, XL], xtR[:, XL], ALU.subtract)
    xrdR_sc.free(xrdR); sch.free(xtR)
    nc.gpsimd.memset(xrR[:, 1026:1028], 0.0)
    xrRS = sch(); nc.sync.dma_start(xrRS[:, XL], xrR[:, 5:1027])
    sch.free(xrR)
    # relu(U), relu(-U): f16 tensor_scalar runs 4x_2p on DVE
    xpU = sch(); nc.vector.tensor_scalar_max(xpU[:, XL], Uf[:, XL], 0.0)
    xpUm = sch(); nc.vector.tensor_scalar(
        xpUm[:, XL], Uf[:, XL], -1.0, 0.0, ALU.mult, ALU.max)
    xaa = sch(); tt(xaa[:, XL], xpU[:, XL], xrL[:, XL], ALU.mult)
    sch.free(xrL, xpU)
    xbb = sch(); tt(xbb[:, XL], xpUm[:, XL], xrRS[:, XL], ALU.mult)
    sch.free(xpUm, xrRS)
    nc.gpsimd.memset(xaa[:, 2:4], 0.0)
    nc.gpsimd.memset(xaa[:, 1026:1028], 0.0)
    nc.gpsimd.memset(xbb[:, 2:4], 0.0)
    nc.gpsimd.memset(xbb[:, 1026:1028], 0.0)

    # ---- divergence combine on PE: psum col j <-> tile col j+4;
    # out[p, j+4] = fn[p-1,j+4] - fn[p,j+4] + fe[p,j+3] - fe[p,j+4]
    # with fn = yaa - ybb, fe = xaa - xbb folded into the bands ----
    pdv = pe_acc(
        [
            (yaa, DFY, 4),
            (ybb, DFYM, 4),
            (xaa, I1F, 3),
            (xbb, IM1F, 3),
            (xaa, IM1F, 4),
            (xbb, I1F, 4),
        ],
        4,
        bsrc=fbands,
    )
    sch.free(yaa, ybb, xaa, xbb)
    act(oc2[:, EY], pdv[:, 0:1024], AF.Copy)
    psc.free(pdv)


def build_nc(units=UNITS_PER_CORE, repeat=1):
    nc = bass.Bass()
    # Square's eps bias rides a const AP.
    _e = nc.alloc_sbuf_tensor("const-f32-eps", [128, 1], F32)
    nc.gpsimd.memset(_e.ap(), WENO_EPS)
    nc.const_aps.aps[(F32, WENO_EPS)] = _e.ap()
    nc.all_engine_barrier()
    h_ext = nc.declare_dram_parameter("h", [units, 128, PX], BF16, isOutput=False)
    u_ext = nc.declare_dram_parameter("u", [units, 128, PX], F16, isOutput=False)
    v_ext = nc.declare_dram_parameter("v", [units, 128, PX], F16, isOutput=False)
    b_ext = nc.declare_dram_parameter(
        "bands", [128, NBANDS * 128], BF16, isOutput=False
    )
    d_ext = nc.declare_dram_parameter(
        "fbands", [128, NFBANDS * 128], F16, isOutput=False
    )
    o_ext = nc.declare_dram_parameter(
        "o", [units, CHUNK, NX - 4], F32, isOutput=True
    )

    with LegalTileContext(nc) as tc:
        with (
            tc.tile_pool(name="inp", bufs=2) as inp,
            tc.tile_pool(name="wk", bufs=2) as wk,
            tc.tile_pool(name="wkf", bufs=1) as wkf,
            tc.tile_pool(name="wkh", bufs=2) as wkh,
            tc.tile_pool(name="outp", bufs=2) as outp,
            tc.tile_pool(name="bnd", bufs=1) as bnd,
            tc.tile_pool(name="ps", bufs=2, space="PSUM") as psum,
        ):
            bands = bnd.tile([128, NBANDS * 128], BF16, tag="bands")
            nc.sync.dma_start(bands[:], b_ext[:])
            fbands = bnd.tile([128, NFBANDS * 128], F16, tag="fbands")
            nc.sync.dma_start(fbands[:], d_ext[:])
            sc = Scratch(wk, [128, W], BF16)
            scf = Scratch(wkf, [128, W], F32, prefix="f")
            sch = Scratch(wkh, [128, W], F16, prefix="h")
            psc = Scratch(psum, [128, 1024], F32, prefix="p", prime=2)
            for _rep in range(repeat):
                for ui in range(units):
                    Q = inp.tile([128, W], BF16, tag="Q")
                    nc.sync.dma_start(Q[:, 2:1028], h_ext[ui, :, :])
                    Uf = inp.tile([128, W], F16, tag="U")
                    nc.sync.dma_start(Uf[:, 2:1028], u_ext[ui, :, :])
                    Vf = inp.tile([128, W], F16, tag="V")
                    nc.sync.dma_start(Vf[:, 2:1028], v_ext[ui, :, :])

                    oc2 = outp.tile([128, W], F32, tag="oc2")
                    _emit_chunk(
                        nc, sc, scf, sch, psc, bands, fbands, Q, Uf, Vf, oc2
                    )
                    # tile col t -> global x = t - 3; rows p in [3..125)
                    nc.sync.dma_start(
                        o_ext[ui, :, :],
                        oc2[3:125, 5:1025],
                    )
    import sys
    print(
        f"build_nc: scratch_tags={sc.n} f16_tags={sch.n} psum_tags={psc.n}",
        file=sys.stderr,
    )
    return nc


_nc_cache = {}


def _get_nc(units=UNITS_PER_CORE, repeat=1, **_ignored):
    key = (units, repeat)
    if key not in _nc_cache:
        _nc_cache[key] = build_nc(units, repeat)
    return _nc_cache[key]


def _units():
    """Flat (z, r0) work list, padded to NCORES * UNITS_PER_CORE."""
    us = [(z, r0) for z in range(1, NZ - 1) for r0 in R0S]
    while len(us) < NCORES * UNITS_PER_CORE:
        us.append(us[-1])
    return us


def make_in_maps(h, u, v):
    import ml_dtypes

    h = np.asarray(h, dtype=np.float32)
    u = np.asarray(u, dtype=np.float32)
    v = np.asarray(v, dtype=np.float32)
    hp = np.pad(h, ((0, 0), (1, 1), (1, 1)), mode="edge").astype(ml_dtypes.bfloat16)
    up = (np.pad(u, ((0, 0), (1, 1), (1, 1)), mode="edge") * np.float32(1.0 / DX)).astype(np.float16)
    vp = (np.pad(v, ((0, 0), (1, 1), (1, 1)), mode="edge") * np.float32(1.0 / DY)).astype(np.float16)
    us = _units()
    bands = make_bands_host()
    fbands = make_fbands_host()
    in_maps = []
    for c in range(NCORES):
        mine = us[c * UNITS_PER_CORE : (c + 1) * UNITS_PER_CORE]
        in_maps.append(
            {
                "h": np.stack([hp[z, r0 : r0 + 128, :] for z, r0 in mine]),
                "u": np.stack([up[z, r0 : r0 + 128, :] for z, r0 in mine]),
                "v": np.stack([vp[z, r0 : r0 + 128, :] for z, r0 in mine]),
                "bands": bands,
                "fbands": fbands,
            }
        )
    return in_maps


def kernel(h, u, v):
    from concourse.bass_utils import run_bass_kernel_spmd

    nc = _get_nc()
    core_ids = list(range(NCORES))
    in_maps = make_in_maps(h, u, v)
    res = run_bass_kernel_spmd(nc, in_maps, core_ids)
    us = _units()
    out = np.zeros((NZ, NY, NX), dtype=np.float32)
    for c in core_ids:
        o = res.results[c]["o"]
        for j, (z, r0) in enumerate(us[c * UNITS_PER_CORE : (c + 1) * UNITS_PER_CORE]):
            out[z, r0 + 2 : r0 + 124, 2 : NX - 2] = o[j]
    return out
